# revision 74
# baseline (speedup 1.0000x reference)
"""SmartLinearAppearance Trainium2 kernel (ragged-gather formulation).

Reference semantics (per (b, n) tracklet, reverse-time scan t = T-1 .. 0):
    xor  = (nv != 0) ^ (v_t != 0)
    prod = nv * v_t
    a_t  = prod * alpha + xor * nv          # per-part coefficient on state
    c_t  = prod * (1 - alpha) + xor * v_t   # per-part coefficient on input
    if m_t: ne = a_t[p] * ne + c_t[p] * e_t ; nv = max(nv, v_t)
    tok = where(any_t m, ne @ W.T + b, 0)

The recurrence is linear in embs given coefficients derived only from
(vis, masks), and it provably only advances on masked steps, so it is
reformulated as a weighted reduction over the masked subsequence:
    ne[n, d] = sum_{s} w[n, s, p(d)] * embs[n, t_s, d]
where t_s enumerates the masked timesteps of tracklet n. Only masked embs
rows are streamed from HBM (dma_gather with host-built indices); with
~50% mask density plus block padding this cuts HBM traffic to ~69%.

Layout: tracklets are packed 3 per 128-slot partition block (sum of 3
masked counts <= 128 w.h.p.; overflow falls back to the full-read kernel).
The coefficient chain runs on the shifted-compacted (slot) axis; one PE
transpose per part turns the [n, slot] weights into the [slot, n]
block-diagonal matmul operand for every block at once.

Sharding: data-parallel over B across the 8 cores; Linear weights
replicated (W pre-transposed to bf16 on the host).
"""

import sys

sys.path.insert(0, "/opt/trn_rl_repo")

import functools

import ml_dtypes
import numpy as np

import concourse.bacc as bacc
import concourse.bass as bass
import concourse.tile as tile
from concourse import mybir, library_config
from concourse.bass_utils import run_bass_kernel_spmd

B, N, T, D, V, TOK = 8, 64, 64, 1792, 7, 512
P = 7          # parts; F = D // P = 256
F = D // P
ALPHA = float(np.float32(0.9))
ONE_MINUS_ALPHA = float(np.float32(1.0) - np.float32(0.9))
DC = D // 128            # 14 d-chunks of 128
S = 128                  # slots per block
SV = S * V               # 896
TPB = 3                  # tracklets per block
NBLK = (N + TPB - 1) // TPB      # 22 blocks
# Blocks per gather call. Each gather has ~2us of fixed Q7 prep cost, so
# fewer/bigger calls keep descriptor generation ahead of the stream; the
# LAST block of each call ends the index array, so its pad slots can be
# trailing -1 (trimmed by the ucode: no descriptors, no HBM traffic).
# Interior blocks pad with row 0 (valid dup reads, weight zero).
GTILES = [1] + [2] * 9 + [1, 1, 1]
assert sum(GTILES) == NBLK
ET_BUFS = 5              # embs gather buffer slots (SBUF rolling pool)

f32 = mybir.dt.float32
bf16 = mybir.dt.bfloat16
i16 = mybir.dt.int16


def _ap(t, offset_elems, dims):
    """Raw AP on a tensor/tile: dims = [[step, count], ...] in elements."""
    base = t[:] if hasattr(t, "shape") else t
    return bass.AP(tensor=base.tensor, offset=base.offset + offset_elems, ap=dims)


def _blk_cols(k):
    return k * TPB, min((k + 1) * TPB, N)


def build_nc_ragged(has_bias=True):
    nc = bacc.Bacc()

    embs_c = nc.dram_tensor("embs_c", [N * T, D], f32, kind="ExternalInput")
    idx_c = nc.dram_tensor("idx_c", [128, NBLK * 128 // 16], i16,
                           kind="ExternalInput")
    visv_c = nc.dram_tensor("visv_c", [N, SV], f32, kind="ExternalInput")
    val_c = nc.dram_tensor("val_c", [N, S], f32, kind="ExternalInput")
    wt_c = nc.dram_tensor("wt_c", [D, TOK], bf16, kind="ExternalInput")
    if has_bias:
        nm_c = nc.dram_tensor("nm_c", [1, N], bf16, kind="ExternalInput")
        b_c = nc.dram_tensor("b_c", [1, TOK], bf16, kind="ExternalInput")
    ident_c = nc.dram_tensor("ident_c", [64, 64], f32, kind="ExternalInput")
    out_c = nc.dram_tensor("out_c", [N, TOK], f32, kind="ExternalOutput")

    with tile.TileContext(nc) as tc:
        with (
            tc.tile_pool(name="small", bufs=1) as small,
            tc.tile_pool(name="et", bufs=ET_BUFS) as etp,
            tc.tile_pool(name="ec", bufs=4) as ecp,
            tc.tile_pool(name="ps", bufs=1, space="PSUM") as ps,
        ):
            nc.gpsimd.load_library(library_config.mlp)

            # idx load first on sync — the gathers are gated on it
            idxs = small.tile([128, NBLK * 128 // 16], i16)
            nc.sync.dma_start(out=idxs, in_=idx_c[:, :])
            visv = small.tile([N, SV], f32)
            nc.sync.dma_start(out=visv, in_=visv_c[:, :])
            val = small.tile([N, S], f32)
            nc.sync.dma_start(out=val, in_=val_c[:, :])
            ident = small.tile([64, 64], f32)
            nc.sync.dma_start(out=ident, in_=ident_c[:, :])
            if has_bias:
                nm_sb = small.tile([1, N], bf16)
                nc.sync.dma_start(out=nm_sb, in_=nm_c[:, :])
                b_sb = small.tile([1, TOK], bf16)
                nc.sync.dma_start(out=b_sb, in_=b_c[:, :])
            wt_sb = small.tile([128, DC, TOK], bf16)
            nc.sync.dma_start(
                out=wt_sb,
                in_=_ap(wt_c, 0, [[TOK, 128], [128 * TOK, DC], [1, TOK]]),
            )

            # ---- gathers (trailing -1 pads of each call cost nothing) ----
            gts = []
            col = 0
            for nt in GTILES:
                ni = nt * 128
                et = etp.tile([128, 2, D], f32)
                nc.gpsimd.dma_gather(
                    et[:, 0:nt, :], embs_c[:, :],
                    idxs[:, col // 16:(col + ni) // 16],
                    ni, ni, D)
                gts.append((et, nt))
                col += ni

            # ---- coefficient chain on [N, 896] (shifted-compact slots) ----
            # host-built pads are zero, so masked vis == visv directly
            mb = bass.AP(tensor=val.tensor, offset=val.offset,
                         ap=[val.ap[0][:], [1, S], [0, V]])

            # exclusive masked suffix max over slots (log-doubling, zero pad)
            PAD = 64 * V
            sA = small.tile([N, SV + PAD], f32)
            sB = small.tile([N, SV + PAD], f32)
            nc.vector.memset(sA, 0.0)
            nc.vector.memset(sB, 0.0)
            nc.vector.tensor_copy(out=sA[:, 0:SV - V], in_=visv[:, V:SV])
            src, dst = sA, sB
            for k in (1, 2, 4, 8, 16, 32, 64):
                nc.vector.tensor_tensor(
                    out=dst[:, 0:SV], in0=src[:, 0:SV],
                    in1=src[:, k * V:k * V + SV], op=mybir.AluOpType.max)
                src, dst = dst, src
            nv = src[:, 0:SV]  # exclusive suffix max, [N, 896]

            n0 = small.tile([N, SV], f32)
            nc.vector.tensor_scalar(out=n0, in0=nv, scalar1=0.0, scalar2=None,
                                    op0=mybir.AluOpType.is_gt)
            v0 = small.tile([N, SV], f32)
            nc.vector.tensor_scalar(out=v0, in0=visv, scalar1=0.0, scalar2=None,
                                    op0=mybir.AluOpType.is_gt)
            xr = small.tile([N, SV], f32)
            nc.vector.tensor_tensor(out=xr, in0=n0, in1=v0,
                                    op=mybir.AluOpType.not_equal)
            prod = small.tile([N, SV], f32)
            nc.vector.tensor_tensor(out=prod, in0=nv, in1=visv,
                                    op=mybir.AluOpType.mult)
            prodf = prod
            xnv = n0  # reuse
            nc.vector.tensor_tensor(out=xnv, in0=xr, in1=nv,
                                    op=mybir.AluOpType.mult)
            av = small.tile([N, SV], f32)
            nc.vector.scalar_tensor_tensor(
                out=av, in0=prodf, scalar=ALPHA, in1=xnv,
                op0=mybir.AluOpType.mult, op1=mybir.AluOpType.add)
            xv = v0  # reuse
            nc.vector.tensor_tensor(out=xv, in0=xr, in1=visv,
                                    op=mybir.AluOpType.mult)
            cc = xr  # reuse
            nc.vector.scalar_tensor_tensor(
                out=cc, in0=prodf, scalar=ONE_MINUS_ALPHA, in1=xv,
                op0=mybir.AluOpType.mult, op1=mybir.AluOpType.add)

            # g = m * (a - 1) + 1, staged with a leading slot of ones
            gb = small.tile([N, SV + V], f32)
            nc.vector.memset(gb[:, 0:V], 1.0)
            av3 = av.rearrange("n (s v) -> n s v", v=V)
            gb3 = _ap(gb, V, [gb.ap[0][:], [V, S], [1, V]])
            nc.vector.scalar_tensor_tensor(
                out=gb3, in0=av3, scalar=1.0, in1=mb,
                op0=mybir.AluOpType.subtract, op1=mybir.AluOpType.mult)
            nc.vector.tensor_scalar(out=gb[:, V:V + SV], in0=gb[:, V:V + SV],
                                    scalar1=1.0, scalar2=None,
                                    op0=mybir.AluOpType.add)

            # exclusive cumulative product over slots per part
            pb = av  # reuse
            for p in range(V):
                dview = _ap(gb, p, [gb.ap[0][:], [V, S]])
                oview = _ap(pb, p, [pb.ap[0][:], [V, S]])
                nc.vector.tensor_tensor_scan(
                    out=oview, data0=dview, data1=dview, initial=1.0,
                    op0=mybir.AluOpType.mult, op1=mybir.AluOpType.bypass)

            # cc is already zero at pad slots (vis=0 there), so mc == cc
            wco = small.tile([N, SV], f32)
            nc.vector.tensor_tensor(out=wco, in0=cc, in1=pb,
                                    op=mybir.AluOpType.mult)

            # ---- block-diagonal weights via PE transpose (per part) ----
            wbdT_ps = ps.tile([128, V, N], f32)
            for p in range(V):
                wview = _ap(wco, p, [wco.ap[0][:], [V, S]])
                nc.tensor.transpose(out=wbdT_ps[:, p, :], in_=wview,
                                    identity=ident[:, :])
            wbdT = small.tile([128, V, N], bf16)
            nc.vector.tensor_copy(out=wbdT, in_=wbdT_ps)

            # ---- stage 1: neT[d, n] = sum_s w[n, s, p(d)] * embs_g[s, d] ----
            # casts alternate ACT/DVE so neither engine gates the stream
            # (DVE is busy with the chain early, so the first few go to ACT)
            # neT is split into two PSUM halves so the dc<7 half can be
            # drained to SBUF (and stage 2 started) while the last block's
            # dc>=7 matmuls still run — keeps the PE busy through the tail.
            HDC = DC // 2
            neT_psA = ps.tile([128, HDC, N], f32)
            neT_psB = ps.tile([128, HDC, N], f32)

            NG = len(GTILES)

            def _cast(gi, et, nt):
                # ACT covers the early gathers (DVE still runs the chain);
                # DVE, 1.6x faster per cast, takes the rest. The final two
                # run on different engines so they drain in parallel.
                ec = ecp.tile([128, 2, D], bf16)
                if gi < 5 or gi == NG - 2:
                    nc.scalar.copy(out=ec[:, 0:nt, :], in_=et[:, 0:nt, :])
                else:
                    nc.vector.tensor_copy(out=ec[:, 0:nt, :],
                                          in_=et[:, 0:nt, :])
                return ec

            def _mm(ecj, k, dc):
                ca, cb = _blk_cols(k)
                t = neT_psA if dc < HDC else neT_psB
                nc.tensor.matmul(
                    out=t[:, dc % HDC, ca:cb],
                    lhsT=ecj[:, dc * 128:(dc + 1) * 128],
                    rhs=wbdT[:, dc // 2, ca:cb],
                    start=True, stop=True)

            k = 0
            for gi, (et, nt) in enumerate(gts[:-1]):
                ec = _cast(gi, et, nt)
                for j in range(nt):
                    for dc in range(DC):
                        _mm(ec[:, j, :], k, dc)
                    k += 1
            et, nt = gts[-1]
            ec = _cast(len(gts) - 1, et, nt)
            for j in range(nt - 1):
                for dc in range(DC):
                    _mm(ec[:, j, :], k, dc)
                k += 1
            for dc in range(HDC):
                _mm(ec[:, nt - 1, :], k, dc)
            neT_sbA = small.tile([128, HDC, N], bf16)
            nc.vector.tensor_copy(out=neT_sbA, in_=neT_psA)
            for dc in range(HDC, DC):
                _mm(ec[:, nt - 1, :], k, dc)
            neT_sbB = small.tile([128, HDC, N], bf16)
            nc.vector.tensor_copy(out=neT_sbB, in_=neT_psB)

            # ---- stage 2: tok[n, k] = sum_d neT[d, n] wt[d, k] + nm[n] b[k] ----
            # two independent PSUM accumulation chains (even/odd dc) hide the
            # per-matmul PSUM read-modify-write serialization; summed on DVE
            tok_ps = ps.tile([N, TOK], f32)
            for dc in range(DC):
                nc.tensor.matmul(
                    out=tok_ps,
                    lhsT=(neT_sbA if dc < HDC else neT_sbB)[:, dc % HDC, :],
                    rhs=wt_sb[:, dc, :],
                    start=(dc == 0),
                    stop=(not has_bias and dc == DC - 1))
            if has_bias:
                nc.tensor.matmul(out=tok_ps, lhsT=nm_sb[0:1, :],
                                 rhs=b_sb[0:1, :], start=False, stop=True)

            tok_sb = small.tile([N, TOK], f32)
            nc.vector.tensor_copy(out=tok_sb, in_=tok_ps)
            nc.sync.dma_start(out=out_c[:, :], in_=tok_sb)

    nc.compile()
    return nc


# ---------------------------------------------------------------------------
# Full-read fallback (baseline kernel) — used if any block overflows.
# ---------------------------------------------------------------------------

NPAIR = N // 2
NGRP = 8
TV = T * V


def build_nc_full():
    nc = bacc.Bacc()

    embs_c = nc.dram_tensor("embs_c", [N, T, D], f32, kind="ExternalInput")
    vis_c = nc.dram_tensor("vis_c", [N, TV], f32, kind="ExternalInput")
    mask_c = nc.dram_tensor("mask_c", [N, T], f32, kind="ExternalInput")
    wt_c = nc.dram_tensor("wt_c", [D, TOK], bf16, kind="ExternalInput")
    bb_c = nc.dram_tensor("bb_c", [N, TOK], f32, kind="ExternalInput")
    out_c = nc.dram_tensor("out_c", [N, TOK], f32, kind="ExternalOutput")

    with tile.TileContext(nc) as tc:
        with (
            tc.tile_pool(name="small", bufs=1) as small,
            tc.tile_pool(name="big", bufs=1) as bigp,
            tc.tile_pool(name="embs", bufs=3) as ep,
            tc.tile_pool(name="ps", bufs=1, space="PSUM") as ps,
            tc.tile_pool(name="dram", bufs=1, space="DRAM") as dram,
        ):
            wt_sb = bigp.tile([128, DC, TOK], bf16)
            nc.gpsimd.dma_start(
                out=wt_sb,
                in_=_ap(wt_c, 0, [[TOK, 128], [128 * TOK, DC], [1, TOK]]),
            )
            bb_sb = small.tile([N, TOK], f32)
            nc.sync.dma_start(out=bb_sb, in_=bb_c[:, :])

            vis = small.tile([N, TV], f32)
            nc.sync.dma_start(out=vis, in_=vis_c[:, :])
            msk = small.tile([N, T], f32)
            nc.sync.dma_start(out=msk, in_=mask_c[:, :])

            mb = bass.AP(tensor=msk.tensor, offset=msk.offset,
                         ap=[msk.ap[0][:], [1, T], [0, V]])
            vis3 = vis.rearrange("n (t v) -> n t v", v=V)

            mv = small.tile([N, T, V], f32)
            nc.vector.tensor_tensor(out=mv, in0=vis3, in1=mb,
                                    op=mybir.AluOpType.mult)
            mvf = mv.rearrange("n t v -> n (t v)")

            PAD = 32 * V
            sA = small.tile([N, TV + PAD], f32)
            sB = small.tile([N, TV + PAD], f32)
            nc.vector.memset(sA, 0.0)
            nc.vector.memset(sB, 0.0)
            nc.vector.tensor_copy(out=sA[:, 0:TV - V], in_=mvf[:, V:TV])
            src, dst = sA, sB
            for k in (1, 2, 4, 8, 16, 32):
                nc.vector.tensor_tensor(
                    out=dst[:, 0:TV], in0=src[:, 0:TV],
                    in1=src[:, k * V:k * V + TV], op=mybir.AluOpType.max)
                src, dst = dst, src
            nv = src[:, 0:TV]

            n0 = small.tile([N, TV], f32)
            nc.vector.tensor_scalar(out=n0, in0=nv, scalar1=0.0, scalar2=None,
                                    op0=mybir.AluOpType.is_gt)
            v0 = small.tile([N, TV], f32)
            nc.vector.tensor_scalar(out=v0, in0=vis, scalar1=0.0, scalar2=None,
                                    op0=mybir.AluOpType.is_gt)
            xr = small.tile([N, TV], f32)
            nc.vector.tensor_tensor(out=xr, in0=n0, in1=v0,
                                    op=mybir.AluOpType.not_equal)
            prod = small.tile([N, TV], f32)
            nc.vector.tensor_tensor(out=prod, in0=nv, in1=vis,
                                    op=mybir.AluOpType.mult)
            xnv = small.tile([N, TV], f32)
            nc.vector.tensor_tensor(out=xnv, in0=xr, in1=nv,
                                    op=mybir.AluOpType.mult)
            av = small.tile([N, TV], f32)
            nc.vector.scalar_tensor_tensor(
                out=av, in0=prod, scalar=ALPHA, in1=xnv,
                op0=mybir.AluOpType.mult, op1=mybir.AluOpType.add)
            xv = small.tile([N, TV], f32)
            nc.vector.tensor_tensor(out=xv, in0=xr, in1=vis,
                                    op=mybir.AluOpType.mult)
            cc = small.tile([N, TV], f32)
            nc.vector.scalar_tensor_tensor(
                out=cc, in0=prod, scalar=ONE_MINUS_ALPHA, in1=xv,
                op0=mybir.AluOpType.mult, op1=mybir.AluOpType.add)

            gb = small.tile([N, TV + V], f32)
            nc.vector.memset(gb[:, 0:V], 1.0)
            av3 = av.rearrange("n (t v) -> n t v", v=V)
            gb3 = _ap(gb, V, [gb.ap[0][:], [V, T], [1, V]])
            nc.vector.scalar_tensor_tensor(
                out=gb3, in0=av3, scalar=1.0, in1=mb,
                op0=mybir.AluOpType.subtract, op1=mybir.AluOpType.mult)
            nc.vector.tensor_scalar(out=gb[:, V:V + TV], in0=gb[:, V:V + TV],
                                    scalar1=1.0, scalar2=None,
                                    op0=mybir.AluOpType.add)

            pb = small.tile([N, TV], f32)
            for p in range(V):
                dview = _ap(gb, p, [gb.ap[0][:], [V, T]])
                oview = _ap(pb, p, [pb.ap[0][:], [V, T]])
                nc.vector.tensor_tensor_scan(
                    out=oview, data0=dview, data1=dview, initial=1.0,
                    op0=mybir.AluOpType.mult, op1=mybir.AluOpType.bypass)

            mc = small.tile([N, T, V], f32)
            nc.vector.tensor_tensor(
                out=mc, in0=cc.rearrange("n (t v) -> n t v", v=V), in1=mb,
                op=mybir.AluOpType.mult)
            wco = small.tile([N, TV], f32)
            nc.vector.tensor_tensor(out=wco, in0=mc.rearrange("n t v -> n (t v)"),
                                    in1=pb, op=mybir.AluOpType.mult)

            nm = small.tile([N, 1], f32)
            nc.vector.tensor_reduce(out=nm, in_=msk, axis=mybir.AxisListType.X,
                                    op=mybir.AluOpType.max)

            w2 = dram.tile([N, TV], f32)
            nc.sync.dma_start(out=w2, in_=wco)
            wbd = small.tile([128, NPAIR, 2, V], bf16)
            nc.vector.memset(wbd, 0.0)
            nc.gpsimd.dma_start(
                out=wbd[0:T, :, 0, :],
                in_=_ap(w2, 0, [[V, T], [2 * TV, NPAIR], [1, V]]))
            nc.gpsimd.dma_start(
                out=wbd[T:128, :, 1, :],
                in_=_ap(w2, TV, [[V, T], [2 * TV, NPAIR], [1, V]]))

            neT_ps = ps.tile([128, DC, N], f32)
            for g in range(NGRP):
                et = ep.tile([128, 4, D], bf16)
                nc.gpsimd.dma_start(
                    out=et[:, :, :],
                    in_=_ap(embs_c, g * 8 * T * D,
                            [[T * D, 2], [D, T], [2 * T * D, 4], [1, D]]))
                for jj in range(4):
                    ip = 4 * g + jj
                    lhs_all = et[:, jj, :]
                    for dc in range(DC):
                        nc.tensor.matmul(
                            out=neT_ps[:, dc, 2 * ip:2 * ip + 2],
                            lhsT=lhs_all[:, dc * 128:(dc + 1) * 128],
                            rhs=wbd[:, ip, :, dc // 2],
                            start=True, stop=True)
            neT_sb = small.tile([128, DC, N], bf16)
            nc.vector.tensor_copy(out=neT_sb, in_=neT_ps)

            tok_ps = ps.tile([N, TOK], f32)
            for dc in range(DC):
                nc.tensor.matmul(
                    out=tok_ps,
                    lhsT=neT_sb[:, dc, :],
                    rhs=wt_sb[:, dc, :],
                    start=(dc == 0), stop=(dc == DC - 1))

            tok_sb = small.tile([N, TOK], f32)
            nc.vector.tensor_add(out=tok_sb, in0=tok_ps, in1=bb_sb)
            nc.vector.tensor_scalar_mul(out=tok_sb, in0=tok_sb, scalar1=nm)
            nc.sync.dma_start(out=out_c[:, :], in_=tok_sb)

    nc.compile()
    return nc


@functools.lru_cache(maxsize=2)
def _get_nc_ragged(has_bias=True):
    return build_nc_ragged(has_bias)


@functools.lru_cache(maxsize=1)
def _get_nc_full():
    return build_nc_full()


def _pack_core(mask_c):
    """mask_c [N, T] bool -> (idx [NBLK*128] int16, per-tracklet (off, cnt))
    or None on block overflow. Interior-block pads read row 0 (weight 0);
    the final block of each gather call pads with trailing -1, which the
    gather ucode trims (no descriptors, no HBM traffic)."""
    idx = np.zeros(NBLK * 128, dtype=np.int16)
    offs = np.zeros(N, dtype=np.int32)
    cnts = np.zeros(N, dtype=np.int32)
    used = np.zeros(NBLK, dtype=np.int32)
    tl = []
    for k in range(NBLK):
        o = 0
        for n in range(k * TPB, min((k + 1) * TPB, N)):
            ts = np.nonzero(mask_c[n])[0]
            c = len(ts)
            if o + c > 128:
                return None
            idx[k * 128 + o:k * 128 + o + c] = n * T + ts
            offs[n], cnts[n] = o, c
            tl.append(ts)
            o += c
        used[k] = o
    # -1 tails (trimmed by the ucode) for the last block of each gather
    # call — but NOT for the first ET_BUFS gathers: their destination SBUF
    # slots hold pre-kernel garbage, and unwritten partitions could decode
    # as NaN/Inf (0-weight * NaN = NaN). Those pad with row-0 dup reads;
    # later slot generations inherit finite embs values, so trimming is safe.
    kend = 0
    for gi, nt in enumerate(GTILES):
        kend += nt
        if gi < ET_BUFS:
            continue
        k = kend - 1
        idx[k * 128 + used[k]:(k + 1) * 128] = -1
    return idx, offs, cnts, tl


def _wrap_idx(idx):
    """[M] -> [128, M/16] int16 wrapped in 16 partitions, replicated 8x."""
    M = idx.shape[0]
    w = idx.reshape(M // 16, 16).T.astype(np.int16)
    return np.ascontiguousarray(np.tile(w, (8, 1)))


def _prep_ragged(embs, vis, masks, W, b, has_bias):
    wt = np.ascontiguousarray(W.T).astype(ml_dtypes.bfloat16)
    bb = b.astype(ml_dtypes.bfloat16).reshape(1, TOK)
    ident = np.eye(64, dtype=np.float32)
    in_maps = []
    for c in range(B):
        packed = _pack_core(np.asarray(masks[c]))
        if packed is None:
            return None
        idx, offs, cnts, tl = packed
        vis_s = np.zeros((N, S, V), dtype=np.float32)
        val_s = np.zeros((N, S), dtype=np.float32)
        for n in range(N):
            o, cn = offs[n], cnts[n]
            if cn:
                vis_s[n, o:o + cn] = vis[c][n, tl[n]]
                val_s[n, o:o + cn] = 1.0
        m = {
            "embs_c": np.ascontiguousarray(
                np.asarray(embs[c]).reshape(N * T, D)),
            "idx_c": _wrap_idx(idx),
            "visv_c": np.ascontiguousarray(vis_s.reshape(N, SV)),
            "val_c": val_s,
            "wt_c": wt,
            "ident_c": ident,
        }
        if has_bias:
            m["nm_c"] = (cnts > 0).astype(ml_dtypes.bfloat16).reshape(1, N)
            m["b_c"] = bb
        in_maps.append(m)
    return in_maps


def _prep_full(embs, vis, masks, W, b):
    wt = np.ascontiguousarray(W.T).astype(ml_dtypes.bfloat16)
    bb = np.ascontiguousarray(np.broadcast_to(
        b.astype(np.float32), (N, TOK)))
    maskf = np.asarray(masks).astype(np.float32)
    in_maps = []
    for c in range(B):
        in_maps.append({
            "embs_c": np.ascontiguousarray(embs[c]),
            "vis_c": np.ascontiguousarray(np.asarray(vis[c]).reshape(N, TV)),
            "mask_c": np.ascontiguousarray(maskf[c]),
            "wt_c": wt,
            "bb_c": bb,
        })
    return in_maps


def run(embs, vis, masks, W, b, **run_kwargs):
    has_bias = bool(np.any(np.asarray(b)))
    in_maps = _prep_ragged(embs, vis, masks, W, b, has_bias)
    if in_maps is not None:
        nc = _get_nc_ragged(has_bias)
    else:
        nc = _get_nc_full()
        in_maps = _prep_full(embs, vis, masks, W, b)
    res = run_bass_kernel_spmd(nc, in_maps, core_ids=list(range(B)),
                               **run_kwargs)
    out = np.stack([res.results[c]["out_c"] for c in range(B)], axis=0)
    return out, res


def kernel(embs, vis, masks, W, b):
    out, _ = run(embs, vis, masks, W, b)
    return out
